# revision 25
# baseline (speedup 1.0000x reference)
"""Trainium2 Bass kernel for nn_ActLayer_49641232007349.

out[b,o] = sum_{i,f} norm(sin(freqs[f]*x[b,i] + phases[f])) * beta[f,o] * lamb[i,o] + bias[o]
with norm(s) = (s - mean_f) / sqrt(eps + var_f), B=8192, I=512, F=64, O=512.

Fast path (rank-R function approximation, data-parallel over batch):
  The 64 normalized scalar functions g_f(t) = c_f*(sin(w_f t + p_f) - m_f)
  are jointly approximated on t in [-5.5, 5.5] (Gaussian-weighted LS) by a
  small shared basis {t, sin(v_1 t), ..., sin(v_R t)}:
      g_f(t) ~= A[0,f]*t + sum_r A[r,f] sin(v_r t)
  Then  out = sum_k U_k @ W_k + bias_eff, where U_0 = x (free!),
  U_r = sin(v_r x), W_k = lamb * gamma_k[None,:], gamma = A @ beta.
  For the harness inputs (phases=0, |w|<=2.64) R=4 optimized nodes give
  max-abs-err/absmax(out) ~= 4.4e-3 (gate is 2e-2), cutting matmul work
  64 -> 5 terms.  A runtime fit-residual check falls back to the exact
  per-frequency path (the proven baseline kernel) if the inputs ever lie
  outside the fitted regime.

Per core (1024 batch rows): only the fp16 copy of x is shipped (the fp16
input quantization adds ~2e-4 rel err, verified on host); the t-term
matmuls start as soon as its first DMA piece lands.  The vector engine
range-reduces (r = rne((v/2pi) x) via the fp16->int16 RNE output convert,
d = (v/2pi) x - r in fp16), the scalar engine evaluates sin(2pi d) in
fp16, and the tensor engine accumulates all 5*4*8 [128x128]x[128x512]
fp16 matmuls in 8 persistent PSUM banks (one per 128-row output block).
Schedule notes, each worth measured ns on hardware:
  - ~34 junk matmuls burn the PE p-state ramp (half clock until ~3us of
    continuous work) while the first DMAs are in flight;
  - input DMA issues run on two queues (Sync: xh pieces, Scalar: w
    pieces) in PE consumption order, since per-issue cost is ~0.6us;
  - the last term runs bank-major so each PSUM bank drains (copy + store
    DMA, alternating engines) while later banks still accumulate, and
    the final bank's drain is split across both engines/queues.
Measured: 52.1-53.6us HW exec (varies ~1us run to run) vs 569us for the
exact per-frequency baseline; rel err 4.44e-3 (gate 2e-2).
"""
import sys
import math

sys.path.insert(0, "/opt/trn_rl_repo")

import numpy as np

import concourse.bacc as bacc
import concourse.mybir as mybir
import concourse.tile as tile
from concourse.bass_utils import run_bass_kernel_spmd

F32 = mybir.dt.float32
F16 = mybir.dt.float16
I16 = mybir.dt.int16

N_CORES = 8
B, I, F, O = 8192, 512, 64, 512
BSH = B // N_CORES          # 1024 batch rows per core
IC = I // 128               # 4 i-chunks
BC = BSH // 128             # 8 b-chunks (one PSUM bank each)
EPS = 1e-3
TWO_PI = 2.0 * math.pi

# Gaussian-weighted-LS-optimized sine nodes for the harness freq set
# (|w| <= 2.6376); larger sets are fallbacks for harder inputs.
NODES_R4 = [0.8188537999112128, 1.3181489042885688,
            2.0607398512478827, 2.5796708660219103]
NODES_R6 = [0.7141618915081405, 1.1524732509852684, 1.5878349218548578,
            2.060601599859188, 2.3931607259574044, 2.6129025082971795]
NODES_R8 = [0.6113729930170966, 0.9581833853886473, 1.280059244458284,
            1.6663806844913058, 2.0148278858066515, 2.281714348438829,
            2.562007683940487, 2.6320924026240533]
WMAX_FIT = 2.6376102
N_WARM_MM = 34      # PE p-state warm-up matmuls (128 cols each)


def _fit_basis(wf, ph, cf, mf, nodes, tmax):
    """Weighted LS fit of g_f(t) = c_f (sin(w_f t + p_f) - m_f) onto
    {t} + {sin(v t + q)}.  Returns (A [(1+R'), F], terms [(v, q)...],
    worst-f weighted-rms residual)."""
    t = np.linspace(-tmax, tmax, 2201)
    gw = np.exp(-0.5 * t * t)
    wfl = 1e-3
    wvec = gw + wfl
    wt = np.sqrt(wvec)
    G = cf[None, :] * (np.sin(np.outer(t, wf) + ph[None, :]) - mf[None, :])
    terms = [(float(v), 0.0) for v in nodes]
    if np.any(np.abs(ph) > 1e-6):
        terms += [(float(v), np.pi / 2) for v in nodes]
    cols = [t[:, None]]
    for v, q in terms:
        cols.append(np.sin(t * v + q)[:, None])
    Phi = np.concatenate(cols, axis=1)
    A, *_ = np.linalg.lstsq(Phi * wt[:, None], G * wt[:, None], rcond=None)
    resid = Phi @ A - G
    wrms = np.sqrt((resid**2 * wvec[:, None]).sum(0) / wvec.sum())
    return A, terms, float(wrms.max())


def _build_rank(terms):
    """Rank-R SPMD module. terms = [(v_r, q_r), ...] sine terms; the
    linear t-term is implicit and runs first off the fp16 x copy."""
    R = len(terms)
    NT = 1 + R
    nc = bacc.Bacc("TRN2", target_bir_lowering=False, debug=False)

    xh = nc.dram_tensor("xh", [128, IC * BSH], F16, kind="ExternalInput").ap()
    w = nc.dram_tensor("w", [NT, 128, IC * O], F16, kind="ExternalInput").ap()
    out = nc.dram_tensor("out", [BSH, O], F32, kind="ExternalOutput").ap()

    sub = mybir.AluOpType.subtract
    mult = mybir.AluOpType.mult
    add = mybir.AluOpType.add
    act_t = mybir.ActivationFunctionType

    with tile.TileContext(nc) as tc:
        with (
            tc.tile_pool(name="xpool", bufs=1) as xpool,
            tc.tile_pool(name="wpool", bufs=4) as wpool,
            tc.tile_pool(name="rpool", bufs=2) as rpool,
            tc.tile_pool(name="spool", bufs=2) as spool,
            tc.tile_pool(name="opool", bufs=2) as opool,
            tc.tile_pool(name="psum", bufs=1, space="PSUM") as pspool,
        ):
            xh_sb = xpool.tile([128, IC * BSH], F16, tag="xh")
            w_tiles = [wpool.tile([128, IC * O], F16, tag="w", name=f"w{k}")
                       for k in range(NT)]

            # DMA issues run on TWO queues in parallel (xh pieces on Sync,
            # weights on the Scalar queue) so the per-issue serialization
            # doesn't gate the t-term's feed rate.
            nc.sync.dma_start(xh_sb[:, 0:512], xh[:, 0:512])
            nc.scalar.dma_start(w_tiles[0][:, 0:O], w[0][:, 0:O])
            nc.sync.dma_start(xh_sb[:, 512:BSH], xh[:, 512:BSH])
            nc.scalar.dma_start(w_tiles[0][:, O:2 * O], w[0][:, O:2 * O])
            nc.sync.dma_start(xh_sb[:, BSH:2 * BSH], xh[:, BSH:2 * BSH])
            nc.scalar.dma_start(w_tiles[0][:, 2 * O:3 * O], w[0][:, 2 * O:3 * O])
            nc.sync.dma_start(xh_sb[:, 2 * BSH:3 * BSH], xh[:, 2 * BSH:3 * BSH])
            nc.scalar.dma_start(w_tiles[0][:, 3 * O:4 * O], w[0][:, 3 * O:4 * O])
            nc.sync.dma_start(xh_sb[:, 3 * BSH:IC * BSH], xh[:, 3 * BSH:IC * BSH])
            for k in range(1, NT):
                nc.scalar.dma_start(w_tiles[k][:], w[k])

            psum_tiles = [
                pspool.tile([128, O], F32, tag=f"ps{bc}", name=f"ps{bc}")
                for bc in range(BC)
            ]

            # PE p-state warm-up: the tensor engine runs at half clock until
            # it has been continuously busy ~3us.  Burn that ramp on junk
            # matmuls while the first input DMAs are still in flight so the
            # real stream starts at full speed.
            junk = opool.tile([128, 128], F16, tag="junk")
            nc.vector.memset(junk[:], 0.0)
            for _ in range(N_WARM_MM):
                nc.tensor.matmul(psum_tiles[0][:, 0:128], lhsT=junk[:],
                                 rhs=junk[:], start=True, stop=False)

            # dummy 1-col Sin up front: hoists the ACT table load into the
            # preamble instead of stalling the first real ACTIVATE
            warm = opool.tile([128, 1], F32, tag="warm")
            nc.vector.memset(warm[:], 0.0)
            nc.scalar.activation(warm[:], warm[:], act_t.Sin, bias=0.0, scale=1.0)

            # term 0: linear t-term straight off xh (no elementwise work)
            for ic in range(IC):
                for bc in range(BC):
                    nc.tensor.matmul(
                        psum_tiles[bc][:],
                        lhsT=xh_sb[:, ic * BSH + bc * 128: ic * BSH + bc * 128 + 128],
                        rhs=w_tiles[0][:, ic * O: (ic + 1) * O],
                        start=(ic == 0),
                        stop=False,
                    )

            for r, (v, q) in enumerate(terms):
                sf = v / TWO_PI
                pf_turn = q / TWO_PI
                last = (r == R - 1)
                w_sb = w_tiles[1 + r]

                rt = rpool.tile([128, IC * BSH], I16, tag="rt")
                dd = rpool.tile([128, IC * BSH], F16, tag="dd")
                ss = spool.tile([128, IC * BSH], F16, tag="ss")
                if r == 0:
                    chunks = [(0, 512), (512, BSH)] + [
                        (ic * BSH, (ic + 1) * BSH) for ic in range(1, IC)]
                else:
                    chunks = [(0, 2 * BSH), (2 * BSH, 4 * BSH)]
                for c0, c1 in chunks:
                    # all-16-bit operands keep the DVE in its fast mode; the
                    # fp16 x adds ~2e-4 relative error (verified on host)
                    nc.vector.tensor_scalar(rt[:, c0:c1], xh_sb[:, c0:c1],
                                            sf, pf_turn, mult, add)
                    nc.vector.scalar_tensor_tensor(dd[:, c0:c1], xh_sb[:, c0:c1],
                                                   sf, rt[:, c0:c1], mult, sub)
                    nc.scalar.activation(ss[:, c0:c1], dd[:, c0:c1], act_t.Sin,
                                         bias=float(q), scale=TWO_PI)

                if not last:
                    for ic in range(IC):
                        for bc in range(BC):
                            nc.tensor.matmul(
                                psum_tiles[bc][:],
                                lhsT=ss[:, ic * BSH + bc * 128: ic * BSH + bc * 128 + 128],
                                rhs=w_sb[:, ic * O: (ic + 1) * O],
                                start=False,
                                stop=False,
                            )
                else:
                    # bank-major so each PSUM bank finishes, drains, and
                    # stores while later banks still accumulate; the store
                    # DMAs go straight from PSUM and are issued from the
                    # vector/scalar queues to avoid serializing on Sync
                    for bc in range(BC):
                        for ic in range(IC):
                            nc.tensor.matmul(
                                psum_tiles[bc][:],
                                lhsT=ss[:, ic * BSH + bc * 128: ic * BSH + bc * 128 + 128],
                                rhs=w_sb[:, ic * O: (ic + 1) * O],
                                start=False,
                                stop=(ic == IC - 1),
                            )
                        ot = opool.tile([128, O], F32, tag=f"ot{bc % 2}")
                        if bc == BC - 1:
                            # final bank is the serial tail: split its drain
                            # across both engines and both DMA queues
                            nc.vector.tensor_copy(ot[:, 0:O // 2],
                                                  psum_tiles[bc][:, 0:O // 2])
                            nc.scalar.copy(ot[:, O // 2:O],
                                           psum_tiles[bc][:, O // 2:O])
                            nc.sync.dma_start(
                                out[bc * 128: (bc + 1) * 128, 0:O // 2],
                                ot[:, 0:O // 2])
                            nc.scalar.dma_start(
                                out[bc * 128: (bc + 1) * 128, O // 2:O],
                                ot[:, O // 2:O])
                        else:
                            if bc % 2 == 0:
                                nc.vector.tensor_copy(ot[:], psum_tiles[bc][:])
                            else:
                                nc.scalar.copy(ot[:], psum_tiles[bc][:])
                            nc.sync.dma_start(out[bc * 128: (bc + 1) * 128, :],
                                              ot[:])

    nc.finalize()
    return nc


def _build_exact(freqs_flat, phases_flat):
    """Exact per-frequency fallback (the proven baseline kernel)."""
    nc = bacc.Bacc("TRN2", target_bir_lowering=False, debug=False)

    xt = nc.dram_tensor("xt", [128, IC * BSH], F32, kind="ExternalInput").ap()
    w = nc.dram_tensor("w", [F, 128, IC * O], F16, kind="ExternalInput").ap()
    bias2 = nc.dram_tensor("bias2", [128, F], F32, kind="ExternalInput").ap()
    out = nc.dram_tensor("out", [BSH, O], F32, kind="ExternalOutput").ap()

    sub = mybir.AluOpType.subtract
    mult = mybir.AluOpType.mult
    add = mybir.AluOpType.add
    act_t = mybir.ActivationFunctionType

    with tile.TileContext(nc) as tc:
        with (
            tc.tile_pool(name="xpool", bufs=1) as xpool,
            tc.tile_pool(name="wpool", bufs=4) as wpool,
            tc.tile_pool(name="rpool", bufs=2) as rpool,
            tc.tile_pool(name="spool", bufs=2) as spool,
            tc.tile_pool(name="opool", bufs=2) as opool,
            tc.tile_pool(name="psum", bufs=1, space="PSUM") as pspool,
        ):
            xt_sb = xpool.tile([128, IC * BSH], F32, tag="xt")
            for ic in range(IC):
                nc.sync.dma_start(xt_sb[:, ic * BSH:(ic + 1) * BSH],
                                  xt[:, ic * BSH:(ic + 1) * BSH])
            b2_sb = opool.tile([128, F], F32, tag="b2")
            nc.sync.dma_start(b2_sb[:], bias2[:])
            warm = opool.tile([128, 1], F32, tag="warm")
            nc.vector.memset(warm[:], 0.0)
            nc.scalar.activation(warm[:], warm[:], act_t.Sin, bias=0.0, scale=1.0)

            psum_tiles = [
                pspool.tile([128, O], F32, tag=f"ps{bc}", name=f"ps{bc}")
                for bc in range(BC)
            ]

            for f in range(F):
                sf = float(freqs_flat[f]) / TWO_PI
                pf_turn = float(phases_flat[f]) / TWO_PI

                w_sb = wpool.tile([128, IC * O], F16, tag="w")
                if f == 0:
                    for ic in range(IC):
                        nc.sync.dma_start(w_sb[:, ic * O:(ic + 1) * O],
                                          w[f][:, ic * O:(ic + 1) * O])
                else:
                    nc.sync.dma_start(w_sb[:], w[f])

                rt = rpool.tile([128, IC * BSH], I16, tag="rt")
                dd = rpool.tile([128, IC * BSH], F32, tag="dd")
                ss = spool.tile([128, IC * BSH], F16, tag="ss")
                if f == 0:
                    chunks = [(ic * BSH, (ic + 1) * BSH) for ic in range(IC)]
                elif f == 1:
                    chunks = [(0, 2 * BSH), (2 * BSH, 4 * BSH)]
                else:
                    chunks = [(0, IC * BSH)]
                for c0, c1 in chunks:
                    nc.vector.tensor_scalar(rt[:, c0:c1], xt_sb[:, c0:c1],
                                            sf, pf_turn, mult, add)
                    nc.vector.scalar_tensor_tensor(dd[:, c0:c1], xt_sb[:, c0:c1],
                                                   sf, rt[:, c0:c1], mult, sub)
                    nc.scalar.activation(ss[:, c0:c1], dd[:, c0:c1], act_t.Sin,
                                         bias=b2_sb[:, f:f + 1], scale=TWO_PI)

                for ic in range(IC):
                    for bc in range(BC):
                        nc.tensor.matmul(
                            psum_tiles[bc][:],
                            lhsT=ss[:, ic * BSH + bc * 128: ic * BSH + bc * 128 + 128],
                            rhs=w_sb[:, ic * O: (ic + 1) * O],
                            start=(f == 0 and ic == 0),
                            stop=(f == F - 1 and ic == IC - 1),
                        )

            for bc in range(BC):
                ot = opool.tile([128, O], F32, tag=f"ot{bc % 2}")
                if bc % 2 == 0:
                    nc.vector.tensor_copy(ot[:], psum_tiles[bc][:])
                else:
                    nc.scalar.copy(ot[:], psum_tiles[bc][:])
                nc.sync.dma_start(out[bc * 128: (bc + 1) * 128, :], ot[:])

    nc.finalize()
    return nc


def _run(nc, in_maps, trace):
    res = None
    for attempt in range(3):
        try:
            res = run_bass_kernel_spmd(nc, in_maps, core_ids=list(range(N_CORES)),
                                       trace=trace)
            break
        except Exception:
            # transient NRT_EXEC_UNIT_UNRECOVERABLE wedges clear on reload
            if attempt == 2:
                raise
            import time as _time
            _time.sleep(5.0)
    return res


def _weight_layout(w_full):
    """[NT, I, O] f64 -> [NT, 128, IC*O] f16 with i = ic*128 + ip -> [ip, ic]."""
    NT = w_full.shape[0]
    wr = w_full.reshape(NT, IC, 128, O).transpose(0, 2, 1, 3)
    return np.ascontiguousarray(wr).reshape(NT, 128, IC * O).astype(np.float16)


def kernel(x, freqs, phases, beta, lamb, bias, _trace=False):
    x = np.ascontiguousarray(x, dtype=np.float32)
    wf = np.asarray(freqs, dtype=np.float64).reshape(-1)      # [F]
    ph = np.asarray(phases, dtype=np.float64).reshape(-1)     # [F]
    beta64 = np.asarray(beta, dtype=np.float64)               # [F, O]
    lamb64 = np.asarray(lamb, dtype=np.float64)               # [I, O]
    bias64 = np.asarray(bias, dtype=np.float64)               # [O]

    # normalization constants (fp64 on host)
    mf = np.exp(-0.5 * wf**2) * np.sin(ph)                               # [F]
    var = 0.5 - 0.5 * np.exp(-2.0 * wf**2) * np.cos(2.0 * ph) - mf**2
    cf = 1.0 / np.sqrt(EPS + var)                                        # [F]

    # rank-1 mean correction folded into the host-side bias
    const_o = (cf * mf) @ beta64 * lamb64.sum(0)
    bias_eff = (bias64 - const_o).astype(np.float32)                     # [O]

    # try the rank-R fast path; fall back to the exact kernel if the
    # runtime fit residual is too large for a comfortable 2e-2 gate
    tmax = max(5.5, float(np.abs(x).max()) + 0.25)
    fit = None
    ratio = float(np.abs(wf).max()) / WMAX_FIT
    wscale = ratio if ratio > 1.001 else 1.0
    for nodes in ([NODES_R4] if wscale == 1.0 and tmax <= 5.6 else []) + [
            NODES_R8,
            [v * wscale for v in NODES_R8],
            list(np.asarray(NODES_R8) * wscale) + list(
                np.asarray(NODES_R4) * 0.5 * wscale)]:
        A, terms, worst = _fit_basis(wf, ph, cf, mf, nodes, tmax)
        if worst < 0.06:
            fit = (A, terms)
            break

    if fit is not None:
        A, terms = fit
        gamma = A @ beta64                                   # [NT, O]
        w_full = lamb64[None, :, :] * gamma[:, None, :]      # [NT, I, O]
        w_host = _weight_layout(w_full)
        nc = _build_rank(terms)
        in_maps = []
        for c in range(N_CORES):
            xs = x[c * BSH: (c + 1) * BSH]                   # [BSH, I]
            xtc = np.ascontiguousarray(
                xs.reshape(BSH, IC, 128).transpose(2, 1, 0).reshape(128, IC * BSH)
            )
            in_maps.append({"xh": xtc.astype(np.float16), "w": w_host})
        res = _run(nc, in_maps, _trace)
    else:
        # exact fallback: per-f weights W_f = lamb * (c_f beta_f)
        w_full = lamb64[None, :, :] * (cf[:, None] * beta64)[:, None, :]
        w_host = _weight_layout(w_full)
        b2 = np.broadcast_to(ph.astype(np.float32), (128, F)).copy()
        nc = _build_exact(wf.astype(np.float32), ph.astype(np.float32))
        in_maps = []
        for c in range(N_CORES):
            xs = x[c * BSH: (c + 1) * BSH]
            xtc = np.ascontiguousarray(
                xs.reshape(BSH, IC, 128).transpose(2, 1, 0).reshape(128, IC * BSH)
            )
            in_maps.append({"xt": xtc, "w": w_host, "bias2": b2})
        res = _run(nc, in_maps, _trace)

    out = np.empty((B, O), dtype=np.float32)
    for c in range(N_CORES):
        out[c * BSH: (c + 1) * BSH] = res.results[c]["out"]
    out += bias_eff[None, :]
    if _trace:
        return out, res
    return out


# revision 26
# speedup vs baseline: 1.0311x; 1.0311x over previous
"""Trainium2 Bass kernel for nn_ActLayer_49641232007349.

out[b,o] = sum_{i,f} norm(sin(freqs[f]*x[b,i] + phases[f])) * beta[f,o] * lamb[i,o] + bias[o]
with norm(s) = (s - mean_f) / sqrt(eps + var_f), B=8192, I=512, F=64, O=512.

Fast path (rank-R function approximation, data-parallel over batch):
  The 64 normalized scalar functions g_f(t) = c_f*(sin(w_f t + p_f) - m_f)
  are jointly approximated on t in [-5.5, 5.5] (Gaussian-weighted LS) by a
  small shared basis {t, sin(v_1 t), ..., sin(v_R t)}:
      g_f(t) ~= A[0,f]*t + sum_r A[r,f] sin(v_r t)
  Then  out = sum_k U_k @ W_k + bias_eff, where U_0 = x (free!),
  U_r = sin(v_r x), W_k = lamb * gamma_k[None,:], gamma = A @ beta.
  For the harness inputs (phases=0, |w|<=2.64) R=4 optimized nodes give
  max-abs-err/absmax(out) ~= 4.4e-3 (gate is 2e-2), cutting matmul work
  64 -> 5 terms.  A runtime fit-residual check falls back to the exact
  per-frequency path (the proven baseline kernel) if the inputs ever lie
  outside the fitted regime.

Per core (1024 batch rows): only the fp16 copy of x is shipped (the fp16
input quantization adds ~2e-4 rel err, verified on host); the t-term
matmuls start as soon as its first DMA piece lands.  The vector engine
range-reduces (r = rne((v/2pi) x) via the fp16->int16 RNE output convert,
d = (v/2pi) x - r in fp16), the scalar engine evaluates sin(2pi d) in
fp16, and the tensor engine accumulates all 5*4*8 [128x128]x[128x512]
fp16 matmuls in 8 persistent PSUM banks (one per 128-row output block).
Schedule notes, each worth measured ns on hardware:
  - ~34 junk matmuls burn the PE p-state ramp (half clock until ~3us of
    continuous work) while the first DMAs are in flight;
  - input DMA issues run on two queues (Sync: xh pieces, Scalar: w
    pieces) in PE consumption order, since per-issue cost is ~0.6us;
  - the last term runs bank-major so each PSUM bank drains (copy + store
    DMA, alternating engines) while later banks still accumulate, and
    the final bank's drain is split across both engines/queues.
Measured: 52.1-53.6us HW exec (varies ~1us run to run) vs 569us for the
exact per-frequency baseline; rel err 4.44e-3 (gate 2e-2).
"""
import sys
import math

sys.path.insert(0, "/opt/trn_rl_repo")

import numpy as np

import concourse.bacc as bacc
import concourse.mybir as mybir
import concourse.tile as tile
from concourse.bass_utils import run_bass_kernel_spmd

F32 = mybir.dt.float32
F16 = mybir.dt.float16
I16 = mybir.dt.int16

N_CORES = 8
B, I, F, O = 8192, 512, 64, 512
BSH = B // N_CORES          # 1024 batch rows per core
IC = I // 128               # 4 i-chunks
BC = BSH // 128             # 8 b-chunks (one PSUM bank each)
EPS = 1e-3
TWO_PI = 2.0 * math.pi

# Gaussian-weighted-LS-optimized sine nodes for the harness freq set
# (|w| <= 2.6376); larger sets are fallbacks for harder inputs.
NODES_R4 = [0.8188537999112128, 1.3181489042885688,
            2.0607398512478827, 2.5796708660219103]
NODES_R6 = [0.7141618915081405, 1.1524732509852684, 1.5878349218548578,
            2.060601599859188, 2.3931607259574044, 2.6129025082971795]
NODES_R8 = [0.6113729930170966, 0.9581833853886473, 1.280059244458284,
            1.6663806844913058, 2.0148278858066515, 2.281714348438829,
            2.562007683940487, 2.6320924026240533]
WMAX_FIT = 2.6376102
N_WARM_MM = 34      # PE p-state warm-up matmuls (128 cols each)


def _fit_basis(wf, ph, cf, mf, nodes, tmax):
    """Weighted LS fit of g_f(t) = c_f (sin(w_f t + p_f) - m_f) onto
    {t} + {sin(v t + q)}.  Returns (A [(1+R'), F], terms [(v, q)...],
    worst-f weighted-rms residual)."""
    t = np.linspace(-tmax, tmax, 2201)
    gw = np.exp(-0.5 * t * t)
    wfl = 1e-3
    wvec = gw + wfl
    wt = np.sqrt(wvec)
    G = cf[None, :] * (np.sin(np.outer(t, wf) + ph[None, :]) - mf[None, :])
    terms = [(float(v), 0.0) for v in nodes]
    if np.any(np.abs(ph) > 1e-6):
        terms += [(float(v), np.pi / 2) for v in nodes]
    cols = [t[:, None]]
    for v, q in terms:
        cols.append(np.sin(t * v + q)[:, None])
    Phi = np.concatenate(cols, axis=1)
    A, *_ = np.linalg.lstsq(Phi * wt[:, None], G * wt[:, None], rcond=None)
    resid = Phi @ A - G
    wrms = np.sqrt((resid**2 * wvec[:, None]).sum(0) / wvec.sum())
    return A, terms, float(wrms.max())


def _build_rank(terms):
    """Rank-R SPMD module. terms = [(v_r, q_r), ...] sine terms; the
    linear t-term is implicit and runs first off the fp16 x copy."""
    R = len(terms)
    NT = 1 + R
    nc = bacc.Bacc("TRN2", target_bir_lowering=False, debug=False)

    xh = nc.dram_tensor("xh", [128, IC * BSH], F16, kind="ExternalInput").ap()
    w = nc.dram_tensor("w", [NT, 128, IC * O], F16, kind="ExternalInput").ap()
    out = nc.dram_tensor("out", [BSH, O], F32, kind="ExternalOutput").ap()

    sub = mybir.AluOpType.subtract
    mult = mybir.AluOpType.mult
    add = mybir.AluOpType.add
    act_t = mybir.ActivationFunctionType

    with tile.TileContext(nc) as tc:
        with (
            tc.tile_pool(name="xpool", bufs=1) as xpool,
            tc.tile_pool(name="wpool", bufs=5) as wpool,
            tc.tile_pool(name="rpool", bufs=2) as rpool,
            tc.tile_pool(name="spool", bufs=2) as spool,
            tc.tile_pool(name="opool", bufs=2) as opool,
            tc.tile_pool(name="psum", bufs=1, space="PSUM") as pspool,
        ):
            xh_sb = xpool.tile([128, IC * BSH], F16, tag="xh")
            w_tiles = [wpool.tile([128, IC * O], F16, tag="w", name=f"w{k}")
                       for k in range(NT)]

            # DMA issues run on TWO queues in parallel (xh pieces on Sync,
            # weights on the Scalar queue) so the per-issue serialization
            # doesn't gate the t-term's feed rate.
            nc.sync.dma_start(xh_sb[:, 0:512], xh[:, 0:512])
            nc.scalar.dma_start(w_tiles[0][:, 0:O], w[0][:, 0:O])
            nc.sync.dma_start(xh_sb[:, 512:BSH], xh[:, 512:BSH])
            nc.scalar.dma_start(w_tiles[0][:, O:2 * O], w[0][:, O:2 * O])
            nc.sync.dma_start(xh_sb[:, BSH:2 * BSH], xh[:, BSH:2 * BSH])
            nc.scalar.dma_start(w_tiles[0][:, 2 * O:3 * O], w[0][:, 2 * O:3 * O])
            nc.sync.dma_start(xh_sb[:, 2 * BSH:3 * BSH], xh[:, 2 * BSH:3 * BSH])
            nc.scalar.dma_start(w_tiles[0][:, 3 * O:4 * O], w[0][:, 3 * O:4 * O])
            nc.sync.dma_start(xh_sb[:, 3 * BSH:IC * BSH], xh[:, 3 * BSH:IC * BSH])
            for k in range(1, NT):
                nc.scalar.dma_start(w_tiles[k][:], w[k])

            psum_tiles = [
                pspool.tile([128, O], F32, tag=f"ps{bc}", name=f"ps{bc}")
                for bc in range(BC)
            ]

            # PE p-state warm-up: the tensor engine runs at half clock until
            # it has been continuously busy ~3us.  Burn that ramp on junk
            # matmuls while the first input DMAs are still in flight so the
            # real stream starts at full speed.
            junk = opool.tile([128, 128], F16, tag="junk")
            nc.vector.memset(junk[:], 0.0)
            for _ in range(N_WARM_MM):
                nc.tensor.matmul(psum_tiles[0][:, 0:128], lhsT=junk[:],
                                 rhs=junk[:], start=True, stop=False)

            # dummy 1-col Sin up front: hoists the ACT table load into the
            # preamble instead of stalling the first real ACTIVATE
            warm = opool.tile([128, 1], F32, tag="warm")
            nc.vector.memset(warm[:], 0.0)
            nc.scalar.activation(warm[:], warm[:], act_t.Sin, bias=0.0, scale=1.0)

            # term 0: linear t-term straight off xh (no elementwise work)
            for ic in range(IC):
                for bc in range(BC):
                    nc.tensor.matmul(
                        psum_tiles[bc][:],
                        lhsT=xh_sb[:, ic * BSH + bc * 128: ic * BSH + bc * 128 + 128],
                        rhs=w_tiles[0][:, ic * O: (ic + 1) * O],
                        start=(ic == 0),
                        stop=False,
                    )

            for r, (v, q) in enumerate(terms):
                sf = v / TWO_PI
                pf_turn = q / TWO_PI
                last = (r == R - 1)
                w_sb = w_tiles[1 + r]

                rt = rpool.tile([128, IC * BSH], I16, tag="rt")
                dd = rpool.tile([128, IC * BSH], F16, tag="dd")
                ss = spool.tile([128, IC * BSH], F16, tag="ss")
                if r == 0:
                    chunks = [(0, 512), (512, BSH)] + [
                        (ic * BSH, (ic + 1) * BSH) for ic in range(1, IC)]
                else:
                    chunks = [(0, 2 * BSH), (2 * BSH, 4 * BSH)]
                for c0, c1 in chunks:
                    # all-16-bit operands keep the DVE in its fast mode; the
                    # fp16 x adds ~2e-4 relative error (verified on host)
                    nc.vector.tensor_scalar(rt[:, c0:c1], xh_sb[:, c0:c1],
                                            sf, pf_turn, mult, add)
                    nc.vector.scalar_tensor_tensor(dd[:, c0:c1], xh_sb[:, c0:c1],
                                                   sf, rt[:, c0:c1], mult, sub)
                    nc.scalar.activation(ss[:, c0:c1], dd[:, c0:c1], act_t.Sin,
                                         bias=float(q), scale=TWO_PI)

                if not last:
                    for ic in range(IC):
                        for bc in range(BC):
                            nc.tensor.matmul(
                                psum_tiles[bc][:],
                                lhsT=ss[:, ic * BSH + bc * 128: ic * BSH + bc * 128 + 128],
                                rhs=w_sb[:, ic * O: (ic + 1) * O],
                                start=False,
                                stop=False,
                            )
                else:
                    # bank-major so each PSUM bank finishes, drains, and
                    # stores while later banks still accumulate; the store
                    # DMAs go straight from PSUM and are issued from the
                    # vector/scalar queues to avoid serializing on Sync
                    for bc in range(BC):
                        for ic in range(IC):
                            nc.tensor.matmul(
                                psum_tiles[bc][:],
                                lhsT=ss[:, ic * BSH + bc * 128: ic * BSH + bc * 128 + 128],
                                rhs=w_sb[:, ic * O: (ic + 1) * O],
                                start=False,
                                stop=(ic == IC - 1),
                            )
                        ot = opool.tile([128, O], F32, tag=f"ot{bc % 2}")
                        if bc == BC - 1:
                            # final bank is the serial tail: split its drain
                            # across both engines and both DMA queues
                            nc.vector.tensor_copy(ot[:, 0:O // 2],
                                                  psum_tiles[bc][:, 0:O // 2])
                            nc.scalar.copy(ot[:, O // 2:O],
                                           psum_tiles[bc][:, O // 2:O])
                            nc.sync.dma_start(
                                out[bc * 128: (bc + 1) * 128, 0:O // 2],
                                ot[:, 0:O // 2])
                            nc.scalar.dma_start(
                                out[bc * 128: (bc + 1) * 128, O // 2:O],
                                ot[:, O // 2:O])
                        else:
                            if bc % 2 == 0:
                                nc.vector.tensor_copy(ot[:], psum_tiles[bc][:])
                            else:
                                nc.scalar.copy(ot[:], psum_tiles[bc][:])
                            nc.sync.dma_start(out[bc * 128: (bc + 1) * 128, :],
                                              ot[:])

    nc.finalize()
    return nc


def _build_exact(freqs_flat, phases_flat):
    """Exact per-frequency fallback (the proven baseline kernel)."""
    nc = bacc.Bacc("TRN2", target_bir_lowering=False, debug=False)

    xt = nc.dram_tensor("xt", [128, IC * BSH], F32, kind="ExternalInput").ap()
    w = nc.dram_tensor("w", [F, 128, IC * O], F16, kind="ExternalInput").ap()
    bias2 = nc.dram_tensor("bias2", [128, F], F32, kind="ExternalInput").ap()
    out = nc.dram_tensor("out", [BSH, O], F32, kind="ExternalOutput").ap()

    sub = mybir.AluOpType.subtract
    mult = mybir.AluOpType.mult
    add = mybir.AluOpType.add
    act_t = mybir.ActivationFunctionType

    with tile.TileContext(nc) as tc:
        with (
            tc.tile_pool(name="xpool", bufs=1) as xpool,
            tc.tile_pool(name="wpool", bufs=5) as wpool,
            tc.tile_pool(name="rpool", bufs=2) as rpool,
            tc.tile_pool(name="spool", bufs=2) as spool,
            tc.tile_pool(name="opool", bufs=2) as opool,
            tc.tile_pool(name="psum", bufs=1, space="PSUM") as pspool,
        ):
            xt_sb = xpool.tile([128, IC * BSH], F32, tag="xt")
            for ic in range(IC):
                nc.sync.dma_start(xt_sb[:, ic * BSH:(ic + 1) * BSH],
                                  xt[:, ic * BSH:(ic + 1) * BSH])
            b2_sb = opool.tile([128, F], F32, tag="b2")
            nc.sync.dma_start(b2_sb[:], bias2[:])
            warm = opool.tile([128, 1], F32, tag="warm")
            nc.vector.memset(warm[:], 0.0)
            nc.scalar.activation(warm[:], warm[:], act_t.Sin, bias=0.0, scale=1.0)

            psum_tiles = [
                pspool.tile([128, O], F32, tag=f"ps{bc}", name=f"ps{bc}")
                for bc in range(BC)
            ]

            for f in range(F):
                sf = float(freqs_flat[f]) / TWO_PI
                pf_turn = float(phases_flat[f]) / TWO_PI

                w_sb = wpool.tile([128, IC * O], F16, tag="w")
                if f == 0:
                    for ic in range(IC):
                        nc.sync.dma_start(w_sb[:, ic * O:(ic + 1) * O],
                                          w[f][:, ic * O:(ic + 1) * O])
                else:
                    nc.sync.dma_start(w_sb[:], w[f])

                rt = rpool.tile([128, IC * BSH], I16, tag="rt")
                dd = rpool.tile([128, IC * BSH], F32, tag="dd")
                ss = spool.tile([128, IC * BSH], F16, tag="ss")
                if f == 0:
                    chunks = [(ic * BSH, (ic + 1) * BSH) for ic in range(IC)]
                elif f == 1:
                    chunks = [(0, 2 * BSH), (2 * BSH, 4 * BSH)]
                else:
                    chunks = [(0, IC * BSH)]
                for c0, c1 in chunks:
                    nc.vector.tensor_scalar(rt[:, c0:c1], xt_sb[:, c0:c1],
                                            sf, pf_turn, mult, add)
                    nc.vector.scalar_tensor_tensor(dd[:, c0:c1], xt_sb[:, c0:c1],
                                                   sf, rt[:, c0:c1], mult, sub)
                    nc.scalar.activation(ss[:, c0:c1], dd[:, c0:c1], act_t.Sin,
                                         bias=b2_sb[:, f:f + 1], scale=TWO_PI)

                for ic in range(IC):
                    for bc in range(BC):
                        nc.tensor.matmul(
                            psum_tiles[bc][:],
                            lhsT=ss[:, ic * BSH + bc * 128: ic * BSH + bc * 128 + 128],
                            rhs=w_sb[:, ic * O: (ic + 1) * O],
                            start=(f == 0 and ic == 0),
                            stop=(f == F - 1 and ic == IC - 1),
                        )

            for bc in range(BC):
                ot = opool.tile([128, O], F32, tag=f"ot{bc % 2}")
                if bc % 2 == 0:
                    nc.vector.tensor_copy(ot[:], psum_tiles[bc][:])
                else:
                    nc.scalar.copy(ot[:], psum_tiles[bc][:])
                nc.sync.dma_start(out[bc * 128: (bc + 1) * 128, :], ot[:])

    nc.finalize()
    return nc


def _run(nc, in_maps, trace):
    res = None
    for attempt in range(3):
        try:
            res = run_bass_kernel_spmd(nc, in_maps, core_ids=list(range(N_CORES)),
                                       trace=trace)
            break
        except Exception:
            # transient NRT_EXEC_UNIT_UNRECOVERABLE wedges clear on reload
            if attempt == 2:
                raise
            import time as _time
            _time.sleep(5.0)
    return res


def _weight_layout(w_full):
    """[NT, I, O] f64 -> [NT, 128, IC*O] f16 with i = ic*128 + ip -> [ip, ic]."""
    NT = w_full.shape[0]
    wr = w_full.reshape(NT, IC, 128, O).transpose(0, 2, 1, 3)
    return np.ascontiguousarray(wr).reshape(NT, 128, IC * O).astype(np.float16)


def kernel(x, freqs, phases, beta, lamb, bias, _trace=False):
    x = np.ascontiguousarray(x, dtype=np.float32)
    wf = np.asarray(freqs, dtype=np.float64).reshape(-1)      # [F]
    ph = np.asarray(phases, dtype=np.float64).reshape(-1)     # [F]
    beta64 = np.asarray(beta, dtype=np.float64)               # [F, O]
    lamb64 = np.asarray(lamb, dtype=np.float64)               # [I, O]
    bias64 = np.asarray(bias, dtype=np.float64)               # [O]

    # normalization constants (fp64 on host)
    mf = np.exp(-0.5 * wf**2) * np.sin(ph)                               # [F]
    var = 0.5 - 0.5 * np.exp(-2.0 * wf**2) * np.cos(2.0 * ph) - mf**2
    cf = 1.0 / np.sqrt(EPS + var)                                        # [F]

    # rank-1 mean correction folded into the host-side bias
    const_o = (cf * mf) @ beta64 * lamb64.sum(0)
    bias_eff = (bias64 - const_o).astype(np.float32)                     # [O]

    # try the rank-R fast path; fall back to the exact kernel if the
    # runtime fit residual is too large for a comfortable 2e-2 gate
    tmax = max(5.5, float(np.abs(x).max()) + 0.25)
    fit = None
    ratio = float(np.abs(wf).max()) / WMAX_FIT
    wscale = ratio if ratio > 1.001 else 1.0
    for nodes in ([NODES_R4] if wscale == 1.0 and tmax <= 5.6 else []) + [
            NODES_R8,
            [v * wscale for v in NODES_R8],
            list(np.asarray(NODES_R8) * wscale) + list(
                np.asarray(NODES_R4) * 0.5 * wscale)]:
        A, terms, worst = _fit_basis(wf, ph, cf, mf, nodes, tmax)
        if worst < 0.06:
            fit = (A, terms)
            break

    if fit is not None:
        A, terms = fit
        gamma = A @ beta64                                   # [NT, O]
        w_full = lamb64[None, :, :] * gamma[:, None, :]      # [NT, I, O]
        w_host = _weight_layout(w_full)
        nc = _build_rank(terms)
        in_maps = []
        for c in range(N_CORES):
            xs = x[c * BSH: (c + 1) * BSH]                   # [BSH, I]
            xtc = np.ascontiguousarray(
                xs.reshape(BSH, IC, 128).transpose(2, 1, 0).reshape(128, IC * BSH)
            )
            in_maps.append({"xh": xtc.astype(np.float16), "w": w_host})
        res = _run(nc, in_maps, _trace)
    else:
        # exact fallback: per-f weights W_f = lamb * (c_f beta_f)
        w_full = lamb64[None, :, :] * (cf[:, None] * beta64)[:, None, :]
        w_host = _weight_layout(w_full)
        b2 = np.broadcast_to(ph.astype(np.float32), (128, F)).copy()
        nc = _build_exact(wf.astype(np.float32), ph.astype(np.float32))
        in_maps = []
        for c in range(N_CORES):
            xs = x[c * BSH: (c + 1) * BSH]
            xtc = np.ascontiguousarray(
                xs.reshape(BSH, IC, 128).transpose(2, 1, 0).reshape(128, IC * BSH)
            )
            in_maps.append({"xt": xtc, "w": w_host, "bias2": b2})
        res = _run(nc, in_maps, _trace)

    out = np.empty((B, O), dtype=np.float32)
    for c in range(N_CORES):
        out[c * BSH: (c + 1) * BSH] = res.results[c]["out"]
    out += bias_eff[None, :]
    if _trace:
        return out, res
    return out
